# revision 1
# baseline (speedup 1.0000x reference)
"""Causal attention kernel for Trainium2 (Bass/Tile), 8-core data-parallel.

Problem: x [8, 2048, 1024] f32; W_query/W_key/W_value [1024, 1024] f32.
    q = x @ Wq; k = x @ Wk; v = x @ Wv       (per batch element)
    out = softmax(causal(q k^T) / 32) @ v

Sharding: batch dim (8) across the 8 NeuronCores, one batch element per
core; each core runs the identical single-core program on its slice.

Per-core program:
  Phase 1 (two 1024-token halves):
    x -> (PE transpose) -> xT[d_in, tok]
    kT[d, m]  = Wk^T x^T   (Wk panel stationary, xT moving)   -> resident
    vT[d, m]  = Wv^T x^T   -> PE transpose -> v[m, d]         -> resident
    qT[d, n]  = Wq^T x^T   -> spilled to DRAM scratch
  Phase 2 (4 chunks of 512 queries):
    load qT chunk; for each 128-query block i:
      S[n, m] tiles via lhsT=qT block, rhs=kT chunks (causal: skip m > n chunks)
      additive causal mask on the diagonal chunk
      expP = exp(S/32) with fused row-sum (denominator partials)
      PE-transpose expP 128x128 blocks -> PT[m, n]
      O[n, d] += PT^T v  accumulated in PSUM over all valid m blocks
      O * (1/denominator) -> DMA out

All matmuls run in float32r (full PE rate for moving dim >= 256,
~1.5e-4 relative error — measured on hw vs 2.3e-3 for bf16).
"""

import os

import numpy as np

# Defensive: recover wedged cores at NRT/PJRT init (no-op on healthy devices).
os.environ.setdefault("NEURON_RT_RESET_CORES", "1")

import concourse.tile as tile
import concourse.mybir as mybir
from concourse import bacc, bass_utils
from concourse.masks import make_identity

F32 = mybir.dt.float32
F32R = mybir.dt.float32r
EXP = mybir.ActivationFunctionType.Exp
AXX = mybir.AxisListType.X

NTOK = 2048      # tokens per batch element (= per core)
D = 1024         # d_in = d_out
P = 128          # partitions
DC = D // P      # 8 d-chunks
NBLK = NTOK // P     # 16 token blocks
NJ = NTOK // 512     # 4 query chunks of 512
NEG = -1.0e9
SCALE = 1.0 / 32.0   # 1/sqrt(D)


def build_program():
    nc = bacc.Bacc("TRN2", target_bir_lowering=False, debug=False,
                   num_devices=8)
    x = nc.dram_tensor("x", [NTOK, D], F32, kind="ExternalInput").ap()
    wq = nc.dram_tensor("W_query", [D, D], F32, kind="ExternalInput").ap()
    wk = nc.dram_tensor("W_key", [D, D], F32, kind="ExternalInput").ap()
    wv = nc.dram_tensor("W_value", [D, D], F32, kind="ExternalInput").ap()
    out = nc.dram_tensor("out", [NTOK, D], F32, kind="ExternalOutput").ap()

    with tile.TileContext(nc) as tc:
        _emit(nc, tc, x, wq, wk, wv, out)
    nc.compile()
    return nc


def _emit(nc, tc, x, wq, wk, wv, out):
    const = tc.alloc_tile_pool(name="const", bufs=1)
    resid = tc.alloc_tile_pool(name="resid", bufs=1)
    ps512 = tc.alloc_tile_pool(name="ps512", bufs=4, space="PSUM")

    # constants: identities for PE transpose, additive causal mask strip
    id32 = const.tile([P, P], F32, tag="id32")
    make_identity(nc, id32)
    id32r = const.tile([P, P], F32R, tag="id32r")
    nc.vector.tensor_copy(id32r, id32)
    # maskA = [0 x384 | causal(128) | NEG x384]; slice width 512 starting at
    # (3-il)*128 puts the causal block at in-chunk block position il.
    maskA = const.tile([P, 896], F32, tag="maskA")
    nc.vector.memset(maskA[:, 0:384], 0.0)
    nc.vector.memset(maskA[:, 384:512], 0.0)
    nc.gpsimd.affine_select(
        out=maskA[:, 384:512], in_=maskA[:, 384:512],
        compare_op=mybir.AluOpType.is_ge, fill=NEG, base=0,
        pattern=[[-1, P]], channel_multiplier=1)
    nc.vector.memset(maskA[:, 512:896], NEG)

    def copy_balanced(sel, out_ap, in_ap):
        # split PSUM->SBUF copy traffic across ACT and DVE
        if sel % 2 == 0:
            nc.scalar.copy(out_ap, in_ap)
        else:
            nc.vector.tensor_copy(out_ap, in_ap)

    # residents
    kT = resid.tile([P, DC, NTOK], F32R, tag="kT")    # [d%128, dchunk, m]
    v = resid.tile([P, NBLK, D], F32R, tag="v")       # [m%128, mblock, d]

    dram = tc.alloc_tile_pool(name="dram", bufs=1, space="DRAM")
    qdram = dram.tile([P, DC, NTOK], F32, tag="qdram")  # qT spill

    # phase-2 PSUM accumulator and denominator pools (allocated up front;
    # they do not overlap the released phase-1 SBUF space)
    pden = tc.alloc_tile_pool(name="pden", bufs=3)
    psbig = tc.alloc_tile_pool(name="psbig", bufs=2, space="PSUM")

    # ---------------- phase 1: projections ----------------
    xnat = tc.alloc_tile_pool(name="xnat", bufs=4)
    wpool = tc.alloc_tile_pool(name="wpool", bufs=3)
    wvpool = tc.alloc_tile_pool(name="wvpool", bufs=2)
    ph1 = tc.alloc_tile_pool(name="ph1", bufs=1)
    ph1b = tc.alloc_tile_pool(name="ph1b", bufs=3)

    def dma_x(tbg, g):
        # half-row tile [128, 512] covering d_in chunks 4g..4g+3 of token
        # block tbg; 4 buffers -> 4 DMAs in flight
        xt = xnat.tile([P, 512], F32R, tag="xt", name="xt")
        nc.sync.dma_start(
            out=xt,
            in_=x[tbg * P:(tbg + 1) * P, g * 512:(g + 1) * 512].bitcast(F32R))
        return xt

    prefetched = {}
    for h in range(2):  # token halves of 1024
        xT = ph1.tile([P, DC, 1024], F32R, tag="xT")  # [din%128, dinchunk, tok]
        for tb in range(8):  # 128-token blocks within half
            tbg = h * 8 + tb
            for g in range(2):
                xt = prefetched.pop((tbg, g), None)
                if xt is None:
                    xt = dma_x(tbg, g)
                trp = ps512.tile([P, 512], F32R, tag="ps512")
                for b4 in range(4):
                    nc.tensor.transpose(
                        trp[:, b4 * P:(b4 + 1) * P],
                        xt[:, b4 * P:(b4 + 1) * P], id32r)
                copy_balanced(
                    g, xT[:, g * 4:(g + 1) * 4, tb * P:(tb + 1) * P],
                    trp.rearrange("p (b f) -> p b f", b=4))

        def weight_pass(iw, w_ap):
            for dout_c in range(DC):
                wpan = wpool.tile([P, DC, P], F32R, tag="wpan")
                nc.sync.dma_start(
                    out=wpan,
                    in_=w_ap[:, dout_c * P:(dout_c + 1) * P]
                    .rearrange("(c p) f -> p c f", p=P).bitcast(F32R))
                for win in range(2):  # 512-token windows within half
                    ps = ps512.tile([P, 512], F32, tag="ps512")
                    for dc in range(DC):
                        nc.tensor.matmul(
                            ps, wpan[:, dc, :],
                            xT[:, dc, win * 512:(win + 1) * 512],
                            start=(dc == 0), stop=(dc == DC - 1))
                    gtok = h * 1024 + win * 512  # global token offset
                    if iw == 1:   # kT resident
                        copy_balanced(dout_c + win,
                                      kT[:, dout_c, gtok:gtok + 512], ps)
                    else:         # qT spill to DRAM
                        qtmp = ph1b.tile([P, 512], F32, tag="qtmp")
                        copy_balanced(dout_c + win + 1, qtmp, ps)
                        nc.sync.dma_start(
                            out=qdram[:, dout_c, gtok:gtok + 512], in_=qtmp)


        weight_pass(0, wq)
        if h == 0:
            # xnat slots are idle during the v-pass; pre-issue the DMAs for
            # the second half's first two token blocks so h1's transposes
            # start immediately when the xT slot frees.
            for pf_g in range(2):
                prefetched[(8, pf_g)] = dma_x(8, pf_g)
                prefetched[(9, pf_g)] = dma_x(9, pf_g)

        # v projection directly in natural [tok, d] layout: xT blocks as the
        # stationary operand, Wv quarter-panels (all 8 d_in chunks x 256
        # d_out) as the moving operand — eliminates the vT transpose pass.
        for q in range(4):  # 256-wide d_out quarters
            wvq = wvpool.tile([P, DC, 256], F32R, tag="wvq")
            nc.sync.dma_start(
                out=wvq,
                in_=wv[:, q * 256:(q + 1) * 256]
                .rearrange("(c p) f -> p c f", p=P).bitcast(F32R))
            for t in range(8):  # 128-token blocks within half
                tg = h * 8 + t  # global token block
                ps = ps512.tile([P, 512], F32, tag="ps512")
                for dc in range(DC):
                    nc.tensor.matmul(
                        ps[:, 0:256], xT[:, dc, t * P:(t + 1) * P],
                        wvq[:, dc, :],
                        start=(dc == 0), stop=(dc == DC - 1))
                copy_balanced(q + t, v[:, tg, q * 256:(q + 1) * 256],
                              ps[:, 0:256])


        weight_pass(1, wk)

    ph1b.release()
    ph1.release()
    wvpool.release()
    wpool.release()
    xnat.release()

    # ---------------- phase 2: attention ----------------
    p2q = tc.alloc_tile_pool(name="p2q", bufs=2)
    pexp = tc.alloc_tile_pool(name="pexp", bufs=4)
    ppt = tc.alloc_tile_pool(name="ppt", bufs=4)
    posb = tc.alloc_tile_pool(name="posb", bufs=2)

    # Work units (j, il, mc) flattened; software-pipelined so the PE
    # transposes+AV of unit u-1 are emitted after the scores of unit u —
    # the ACT exp of u-1 then hides behind u's score matmuls.
    units = []
    for j in range(NJ):
        for il in range(4):
            for mc in range(j + 1):
                units.append((j, il, mc))

    state = {}  # per-(j,il) live tiles: Ops, dpart

    def emit_scores(u):
        j, il, mc = u
        qTj, _ = qstate[j]
        key = (j, il)
        if key not in state:
            state[key] = (psbig.tile([P, D], F32, tag="psbig", name="Ops"),
                          pden.tile([P, 6], F32, tag="dpart", name="dpart"))
        Ops, dpart = state[key]
        diag = (mc == j)
        # width of the valid score region in this chunk; keep >= 256 so
        # the fp32r matmul stays at full rate
        wv_ = max((il + 1) * P, 256) if diag else 512
        sS = ps512.tile([P, 512], F32, tag="ps512")
        for dc in range(DC):
            nc.tensor.matmul(
                sS[:, 0:wv_], qTj[:, dc, il * P:(il + 1) * P],
                kT[:, dc, mc * 512:mc * 512 + wv_],
                start=(dc == 0), stop=(dc == DC - 1))
        if diag:
            s0 = (3 - il) * P
            nc.vector.tensor_add(sS[:, 0:wv_], sS[:, 0:wv_],
                                 maskA[:, s0:s0 + wv_])
        expP = pexp.tile([P, 512], F32R, tag="expP")
        nc.scalar.activation(expP[:, 0:wv_], sS[:, 0:wv_], EXP, scale=SCALE,
                             accum_out=dpart[:, mc:mc + 1])
        return expP

    def emit_av(u, expP):
        j, il, mc = u
        Ops, dpart = state[(j, il)]
        nb = il + 1 if mc == j else 4
        ptp = ps512.tile([P, 512], F32R, tag="ps512")
        for b in range(nb):
            nc.tensor.transpose(
                ptp[:, b * P:(b + 1) * P],
                expP[:, b * P:(b + 1) * P], id32r)
        PT = ppt.tile([P, 512], F32R, tag="PT")
        copy_balanced(4 * u[0] + u[2], PT[:, 0:nb * P], ptp[:, 0:nb * P])
        for b in range(nb):
            mb = 4 * mc + b
            last = (mc == j and b == nb - 1)
            for hf in range(2):
                nc.tensor.matmul(
                    Ops[:, hf * 512:(hf + 1) * 512],
                    PT[:, b * P:(b + 1) * P],
                    v[:, mb, hf * 512:(hf + 1) * 512],
                    start=(mc == 0 and b == 0), stop=last)
        if last:  # finish query block i = 4j + il
            i = 4 * j + il
            den = pden.tile([P, 2], F32, tag="den")
            nc.vector.reduce_sum(out=den[:, 0:1], in_=dpart[:, 0:j + 1], axis=AXX)
            nc.vector.reciprocal(den[:, 1:2], den[:, 0:1])
            Osb = posb.tile([P, D], F32, tag="Osb")
            nc.vector.tensor_scalar_mul(Osb, Ops, den[:, 1:2])
            nc.sync.dma_start(out=out[i * P:(i + 1) * P, :], in_=Osb)
            del state[(j, il)]

    qstate = {}
    prev = None  # (unit, expP)
    for u in units:
        j = u[0]
        if j not in qstate:
            qTj = p2q.tile([P, DC, 512], F32R, tag="qTj")
            nc.sync.dma_start(
                out=qTj,
                in_=qdram[:, :, j * 512:(j + 1) * 512].bitcast(F32R))
            qstate[j] = (qTj, True)
        expP = emit_scores(u)
        if prev is not None:
            emit_av(*prev)
        prev = (u, expP)
    emit_av(*prev)

    for pool in (posb, ppt, pexp, p2q, psbig, pden, dram, ps512,
                 resid, const):
        pool.release()


_NC_CACHE = None


def _get_nc():
    global _NC_CACHE
    if _NC_CACHE is None:
        _NC_CACHE = build_program()
    return _NC_CACHE


def kernel(x, W_query, W_key, W_value):
    """Full causal attention: x [8, 2048, 1024] -> [8, 2048, 1024] (f32)."""
    nc = _get_nc()
    x = np.ascontiguousarray(np.asarray(x, dtype=np.float32))
    wq = np.ascontiguousarray(np.asarray(W_query, dtype=np.float32))
    wk = np.ascontiguousarray(np.asarray(W_key, dtype=np.float32))
    wv = np.ascontiguousarray(np.asarray(W_value, dtype=np.float32))
    n_cores = x.shape[0]
    in_maps = [
        {"x": x[b], "W_query": wq, "W_key": wk, "W_value": wv}
        for b in range(n_cores)
    ]
    res = bass_utils.run_bass_kernel_spmd(nc, in_maps, core_ids=list(range(n_cores)))
    return np.stack([res.results[b]["out"] for b in range(n_cores)])



# revision 2
# speedup vs baseline: 1.0005x; 1.0005x over previous
"""Causal attention kernel for Trainium2 (Bass/Tile), 8-core data-parallel.

Problem: x [8, 2048, 1024] f32; W_query/W_key/W_value [1024, 1024] f32.
    q = x @ Wq; k = x @ Wk; v = x @ Wv       (per batch element)
    out = softmax(causal(q k^T) / 32) @ v

Sharding: batch dim (8) across the 8 NeuronCores, one batch element per
core; each core runs the identical single-core program on its slice.

Per-core program (v2 — streaming chunks, transposed-score layout):
  Two 1024-token chunks c. Per chunk:
    x_c -> (PE transpose) -> xT[d_in, tok]     (win-interleaved with kT)
    kT[d, m] = Wk^T x^T   -> resident (full seq)
    qT[d, n] = Wq^T x^T   -> chunk-local (no DRAM spill)
    v [m, d] = x W_v      -> resident bf16
    For each 512-query chunk j in c:
      S^T[m, n] tiles via lhsT=kT block, rhs=qT  (no P transpose needed:
        exp(S^T) blocks feed the AV matmul directly as stationary operands)
      additive causal mask on diagonal tiles; exp via ACT -> expP bf16
      O[n, d] (+ denominator in PSUM cols 1024:1032 via ones-matmul)
        accumulated over all valid m blocks; scale by 1/den -> DMA out

Matmuls: projections+scores in float32r (full PE rate, moving >= 256);
AV in bf16 (1 cyc/row at any width; P and v quantization ~1.5e-3 rel err).
"""

import os

import numpy as np

# Defensive: recover wedged cores at NRT/PJRT init (no-op on healthy devices).
os.environ.setdefault("NEURON_RT_RESET_CORES", "1")

import concourse.tile as tile
import concourse.mybir as mybir
from concourse import bacc, bass_utils
from concourse.masks import make_identity

F32 = mybir.dt.float32
F32R = mybir.dt.float32r
BF16 = mybir.dt.bfloat16
EXP = mybir.ActivationFunctionType.Exp

NTOK = 2048      # tokens per batch element (= per core)
D = 1024         # d_in = d_out
P = 128          # partitions
DC = D // P      # 8 d-chunks
NBLK = NTOK // P     # 16 token blocks
NEG = -1.0e9
SCALE = 1.0 / 32.0   # 1/sqrt(D)


def build_program():
    nc = bacc.Bacc("TRN2", target_bir_lowering=False, debug=False,
                   num_devices=8)
    x = nc.dram_tensor("x", [NTOK, D], F32, kind="ExternalInput").ap()
    wq = nc.dram_tensor("W_query", [D, D], F32, kind="ExternalInput").ap()
    wk = nc.dram_tensor("W_key", [D, D], F32, kind="ExternalInput").ap()
    wv = nc.dram_tensor("W_value", [D, D], F32, kind="ExternalInput").ap()
    out = nc.dram_tensor("out", [NTOK, D], F32, kind="ExternalOutput").ap()

    with tile.TileContext(nc) as tc:
        _emit(nc, tc, x, wq, wk, wv, out)
    nc.compile()
    return nc


def _emit(nc, tc, x, wq, wk, wv, out):
    const = tc.alloc_tile_pool(name="const", bufs=1)
    resid = tc.alloc_tile_pool(name="resid", bufs=1)
    ps512 = tc.alloc_tile_pool(name="ps512", bufs=2, space="PSUM")
    psbig = tc.alloc_tile_pool(name="psbig", bufs=2, space="PSUM")
    xnat = tc.alloc_tile_pool(name="xnat", bufs=6)
    pden = tc.alloc_tile_pool(name="pden", bufs=3)
    posb = tc.alloc_tile_pool(name="posb", bufs=2)

    # constants
    id32 = const.tile([P, P], F32, tag="id32")
    make_identity(nc, id32)
    id32r = const.tile([P, P], F32R, tag="id32r")
    nc.vector.tensor_copy(id32r, id32)
    # maskS[r, c] = 0 where c >= r + 128 else NEG.  Diagonal score tile
    # (key block il' within its query chunk, moving window starts at nlo):
    # slice [off : off+w] with off = 128 - (il'*128 - nlo) in {0, 128}.
    maskS = const.tile([P, 640], F32, tag="maskS")
    nc.vector.memset(maskS, 0.0)
    nc.gpsimd.affine_select(
        out=maskS, in_=maskS,
        compare_op=mybir.AluOpType.is_ge, fill=NEG, base=-P,
        pattern=[[1, 640]], channel_multiplier=-1)
    ones8 = const.tile([P, 8], BF16, tag="ones8")
    nc.vector.memset(ones8, 1.0)

    # residents
    kT = resid.tile([P, DC, NTOK], F32R, tag="kT")    # [d%128, dchunk, m]
    vres = resid.tile([P, NBLK, D], BF16, tag="vres")  # [m%128, mblock, d]

    def copy_balanced(sel, out_ap, in_ap):
        # split PSUM->SBUF copy traffic across ACT and DVE
        if sel % 2 == 0:
            nc.scalar.copy(out_ap, in_ap)
        else:
            nc.vector.tensor_copy(out_ap, in_ap)

    def dma_x(c, tb, g):
        # [128 tok, 512 d_in] tile: token block tb of chunk c, d_in half g
        xt = xnat.tile([P, 512], F32R, tag="xt", name="xt")
        tbg = c * 8 + tb
        nc.sync.dma_start(
            out=xt,
            in_=x[tbg * P:(tbg + 1) * P, g * 512:(g + 1) * 512].bitcast(F32R))
        return xt

    def dma_wpan(w_ap, p):
        # weight panel for dout block p: [din%128, dinchunk, dout 128]
        wpan = wpool.tile([P, DC, P], F32R, tag="wpan", name="wpan")
        nc.sync.dma_start(
            out=wpan,
            in_=w_ap[:, p * P:(p + 1) * P]
            .rearrange("(c p) f -> p c f", p=P).bitcast(F32R))
        return wpan

    for c in range(2):
        qpool = tc.alloc_tile_pool(name="qpool", bufs=1)
        xpool = tc.alloc_tile_pool(name="xpool", bufs=1)
        wpool = tc.alloc_tile_pool(name="wpool", bufs=2)
        wvpool = tc.alloc_tile_pool(name="wvpool", bufs=2)

        qTc = qpool.tile([P, DC, 1024], F32R, tag="qTc")  # [d%128, dchunk, n]
        xT = xpool.tile([P, DC, 1024], F32R, tag="xT")    # [din%128, chunk, tok]

        # -- kT pass, win-interleaved with x transposes (first x consumer) --
        for win in range(2):
            for tb in range(win * 4, win * 4 + 4):
                for g in range(2):
                    xt = dma_x(c, tb, g)
                    trp = ps512.tile([P, 512], F32R, tag="ps512", name="trp")
                    for b4 in range(4):
                        nc.tensor.transpose(
                            trp[:, b4 * P:(b4 + 1) * P],
                            xt[:, b4 * P:(b4 + 1) * P], id32r)
                    copy_balanced(
                        g, xT[:, g * 4:(g + 1) * 4, tb * P:(tb + 1) * P],
                        trp.rearrange("p (b f) -> p b f", b=4))
            for p in range(DC):
                wpan = dma_wpan(wk, p)
                ps = ps512.tile([P, 512], F32, tag="ps512", name="psk")
                for dc in range(DC):
                    nc.tensor.matmul(
                        ps, wpan[:, dc, :],
                        xT[:, dc, win * 512:(win + 1) * 512],
                        start=(dc == 0), stop=(dc == DC - 1))
                gtok = c * 1024 + win * 512
                copy_balanced(p + win, kT[:, p, gtok:gtok + 512], ps)

        # -- qT pass (panel-wise; xT fully resident now) --
        for p in range(DC):
            wpan = dma_wpan(wq, p)
            for win in range(2):
                ps = ps512.tile([P, 512], F32, tag="ps512", name="psq")
                for dc in range(DC):
                    nc.tensor.matmul(
                        ps, wpan[:, dc, :],
                        xT[:, dc, win * 512:(win + 1) * 512],
                        start=(dc == 0), stop=(dc == DC - 1))
                copy_balanced(p + win + 1,
                              qTc[:, p, win * 512:(win + 1) * 512], ps)

        # -- v pass: natural [tok, d] layout, xT stationary, Wv moving --
        for q4 in range(4):
            wvq = wvpool.tile([P, DC, 256], F32R, tag="wvq", name="wvq")
            nc.sync.dma_start(
                out=wvq,
                in_=wv[:, q4 * 256:(q4 + 1) * 256]
                .rearrange("(c p) f -> p c f", p=P).bitcast(F32R))
            for t in range(8):
                ps = ps512.tile([P, 512], F32, tag="ps512", name="psv")
                for dc in range(DC):
                    nc.tensor.matmul(
                        ps[:, 0:256], xT[:, dc, t * P:(t + 1) * P],
                        wvq[:, dc, :],
                        start=(dc == 0), stop=(dc == DC - 1))
                copy_balanced(q4 + t, vres[:, c * 8 + t, q4 * 256:(q4 + 1) * 256],
                              ps[:, 0:256])

        wvpool.release()
        wpool.release()
        xpool.release()

        # -- attention for the two 512-query chunks of this chunk --
        apool = tc.alloc_tile_pool(name="apool", bufs=1)
        expP = apool.tile([P, NBLK, 512], BF16, tag="expP")  # [m%128, mblk, n]

        for jj in range(2):
            j = 2 * c + jj
            # scores S^T[m, n] + exp, all key blocks mb for query chunk j
            for mb in range(4 * j + 4):
                il_p = mb - 4 * j  # >= 0 on the diagonal 512-chunk
                nlo = min(il_p * P, 256) if il_p >= 0 else 0
                w = 512 - nlo
                sS = ps512.tile([P, 512], F32, tag="ps512", name="sS")
                for dc in range(DC):
                    nc.tensor.matmul(
                        sS[:, 0:w], kT[:, dc, mb * P:(mb + 1) * P],
                        qTc[:, dc, jj * 512 + nlo:(jj + 1) * 512],
                        start=(dc == 0), stop=(dc == DC - 1))
                if il_p >= 0:
                    off = 0 if il_p == 3 else P
                    nc.vector.tensor_add(sS[:, 0:w], sS[:, 0:w],
                                         maskS[:, off:off + w])
                nc.scalar.activation(expP[:, mb, nlo:512], sS[:, 0:w], EXP,
                                     scale=SCALE)
            # AV accumulation per 128-query block i (den rides in cols
            # 1024:1032 of the same PSUM tile via the ones8 matmul)
            for il in range(4):
                i = 4 * j + il
                Ops = psbig.tile([P, 1536], F32, tag="psbig", name="Ops")
                for mb in range(i + 1):
                    st = (mb == 0)
                    sp = (mb == i)
                    lhsT = expP[:, mb, il * P:(il + 1) * P]
                    nc.tensor.matmul(Ops[:, 0:512], lhsT,
                                     vres[:, mb, 0:512], start=st, stop=sp)
                    nc.tensor.matmul(Ops[:, 512:1024], lhsT,
                                     vres[:, mb, 512:1024], start=st, stop=sp)
                    nc.tensor.matmul(Ops[:, 1024:1032], lhsT, ones8,
                                     start=st, stop=sp)
                rcp = pden.tile([P, 1], F32, tag="rcp", name="rcp")
                nc.vector.reciprocal(rcp, Ops[:, 1024:1025])
                Osb = posb.tile([P, D], F32, tag="Osb", name="Osb")
                nc.vector.tensor_scalar_mul(Osb, Ops[:, 0:D], rcp)
                nc.sync.dma_start(out=out[i * P:(i + 1) * P, :], in_=Osb)

        apool.release()
        qpool.release()

    for pool in (posb, pden, xnat, psbig, ps512, resid, const):
        pool.release()


_NC_CACHE = None


def _get_nc():
    global _NC_CACHE
    if _NC_CACHE is None:
        _NC_CACHE = build_program()
    return _NC_CACHE


def kernel(x, W_query, W_key, W_value):
    """Full causal attention: x [8, 2048, 1024] -> [8, 2048, 1024] (f32)."""
    nc = _get_nc()
    x = np.ascontiguousarray(np.asarray(x, dtype=np.float32))
    wq = np.ascontiguousarray(np.asarray(W_query, dtype=np.float32))
    wk = np.ascontiguousarray(np.asarray(W_key, dtype=np.float32))
    wv = np.ascontiguousarray(np.asarray(W_value, dtype=np.float32))
    n_cores = x.shape[0]
    in_maps = [
        {"x": x[b], "W_query": wq, "W_key": wk, "W_value": wv}
        for b in range(n_cores)
    ]
    res = bass_utils.run_bass_kernel_spmd(nc, in_maps, core_ids=list(range(n_cores)))
    return np.stack([res.results[b]["out"] for b in range(n_cores)])


# revision 17
# speedup vs baseline: 1.1261x; 1.1256x over previous
"""Causal attention kernel for Trainium2 (Bass/Tile), 8-core data-parallel.

Problem: x [8, 2048, 1024] f32; W_query/W_key/W_value [1024, 1024] f32.
    q = x @ Wq; k = x @ Wk; v = x @ Wv       (per batch element)
    out = softmax(causal(q k^T) / 32) @ v

Sharding: batch dim (8) across the 8 NeuronCores, one batch element per
core; each core runs the identical single-core program on its slice.

Per-core program (v2 — streaming chunks, transposed-score layout):
  Two 1024-token chunks c. Per chunk:
    x_c -> (PE transpose) -> xT[d_in, tok]     (win-interleaved with kT)
    kT[d, m] = Wk^T x^T   -> resident (full seq)
    qT[d, n] = Wq^T x^T   -> chunk-local (no DRAM spill)
    v [m, d] = x W_v      -> resident bf16
    For each 512-query chunk j in c:
      S^T[m, n] tiles via lhsT=kT block, rhs=qT  (no P transpose needed:
        exp(S^T) blocks feed the AV matmul directly as stationary operands)
      additive causal mask on diagonal tiles; exp via ACT -> expP bf16
      O[n, d] (+ denominator in PSUM cols 1024:1032 via ones-matmul)
        accumulated over all valid m blocks; scale by 1/den -> DMA out

Matmuls: projections+scores in float32r (full PE rate, moving >= 256);
AV in bf16 (1 cyc/row at any width; P and v quantization ~1.5e-3 rel err).
"""

import os

import numpy as np

# Defensive: recover wedged cores at NRT/PJRT init (no-op on healthy devices).
os.environ.setdefault("NEURON_RT_RESET_CORES", "1")

import concourse.tile as tile
import concourse.mybir as mybir
from concourse import bacc, bass_utils
from concourse.masks import make_identity

F32 = mybir.dt.float32
F32R = mybir.dt.float32r
BF16 = mybir.dt.bfloat16
EXP = mybir.ActivationFunctionType.Exp

NTOK = 2048      # tokens per batch element (= per core)
D = 1024         # d_in = d_out
P = 128          # partitions
DC = D // P      # 8 d-chunks
NBLK = NTOK // P     # 16 token blocks
NEG = -1.0e9
SCALE = 1.0 / 32.0   # 1/sqrt(D)


def build_program():
    nc = bacc.Bacc("TRN2", target_bir_lowering=False, debug=False,
                   num_devices=8)
    x = nc.dram_tensor("x", [NTOK, D], F32, kind="ExternalInput").ap()
    wq = nc.dram_tensor("W_query", [D, D], F32, kind="ExternalInput").ap()
    wk = nc.dram_tensor("W_key", [D, D], F32, kind="ExternalInput").ap()
    wv = nc.dram_tensor("W_value", [D, D], F32, kind="ExternalInput").ap()
    out = nc.dram_tensor("out", [NTOK, D], F32, kind="ExternalOutput").ap()

    with tile.TileContext(nc) as tc:
        _emit(nc, tc, x, wq, wk, wv, out)
    nc.compile()
    return nc


def _emit(nc, tc, x, wq, wk, wv, out):
    const = tc.alloc_tile_pool(name="const", bufs=1)
    resid = tc.alloc_tile_pool(name="resid", bufs=1)
    ps512 = tc.alloc_tile_pool(name="ps512", bufs=3, space="PSUM")
    psbig = tc.alloc_tile_pool(name="psbig", bufs=2, space="PSUM")
    pdenb = tc.alloc_tile_pool(name="pdenb", bufs=1, space="PSUM")
    xnat = tc.alloc_tile_pool(name="xnat", bufs=8)
    pden = tc.alloc_tile_pool(name="pden", bufs=3)
    posb = tc.alloc_tile_pool(name="posb", bufs=2)

    # constants
    id32 = const.tile([P, P], F32, tag="id32")
    make_identity(nc, id32)
    id32r = const.tile([P, P], F32R, tag="id32r")
    nc.vector.tensor_copy(id32r, id32)
    # maskS[r, c] = 0 where c >= r else NEG.  Diagonal score tile for key
    # block il' starts its moving window at nlo = il'*128, so local col
    # c = n - nlo and causal validity n >= il'*128 + r becomes c >= r.
    maskS = const.tile([P, 512], F32, tag="maskS")
    nc.vector.memset(maskS, 0.0)
    nc.gpsimd.affine_select(
        out=maskS, in_=maskS,
        compare_op=mybir.AluOpType.is_ge, fill=NEG, base=0,
        pattern=[[1, 512]], channel_multiplier=-1)
    ones8 = const.tile([P, 8], BF16, tag="ones8")
    nc.vector.memset(ones8, 1.0)

    # residents
    kT = resid.tile([P, DC, NTOK], BF16, tag="kT")    # [d%128, dchunk, m]
    vres = resid.tile([P, NBLK, D], BF16, tag="vres")  # [m%128, mblock, d]

    def copy_balanced(sel, out_ap, in_ap):
        # split PSUM->SBUF copy traffic across ACT and DVE
        if sel % 2 == 0:
            nc.scalar.copy(out_ap, in_ap)
        else:
            nc.vector.tensor_copy(out_ap, in_ap)

    def dma_x(c, tb, split=1):
        # [128 tok, 1024 d_in] tile: full token block tb of chunk c.  The
        # 8-deep ring holds a whole chunk, so chunk 1's loads all dispatch
        # during chunk-0 attention with no SP queue blocking.  split>1 lands
        # the tile in pieces so the very first transpose starts sooner.
        xt = xnat.tile([P, 1024], F32R, tag="xt", name="xt")
        tbg = c * 8 + tb
        w = 1024 // split
        for s in range(split):
            nc.sync.dma_start(
                out=xt[:, s * w:(s + 1) * w],
                in_=x[tbg * P:(tbg + 1) * P, s * w:(s + 1) * w].bitcast(F32R))
        return xt

    def dma_wpan_half(w_ap, p, h):
        # half weight panel: d_in rows h*512..h*512+511 of dout block p,
        # as [din%128, dinchunk h*4.., dout 128].  Halves keep the DMA ring
        # deep enough (5 bufs) to hide the ~3.6us dispatch+transfer latency
        # behind the ~1.7us/panel matmul rate of the kT win pass.
        wpan = wpool.tile([P, 4, P], F32R, tag="wpan", name="wpan")
        nc.sync.dma_start(
            out=wpan,
            in_=w_ap[h * 512:(h + 1) * 512, p * P:(p + 1) * P]
            .rearrange("(c p) f -> p c f", p=P).bitcast(F32R))
        return wpan

    denb = pdenb.tile([P, 512], F32, tag="denb", name="denb")

    for c in range(2):
        qpool = tc.alloc_tile_pool(name="qpool", bufs=1)
        xpool = tc.alloc_tile_pool(name="xpool", bufs=1)
        wpool = tc.alloc_tile_pool(name="wpool", bufs=12)
        wvpool = tc.alloc_tile_pool(name="wvpool", bufs=2)

        qTc = qpool.tile([P, DC, 1024], BF16, tag="qTc")  # [d%128, dchunk, n]
        xT = xpool.tile([P, DC, 1024], F32R, tag="xT")    # [din%128, chunk, tok]

        def transp_win(win, xts):
            for tb in range(win * 4, win * 4 + 4):
                xt = xts[tb - win * 4]
                for g in range(2):
                    trp = ps512.tile([P, 512], F32R, tag="ps512", name="trp")
                    for b4 in range(4):
                        nc.tensor.transpose(
                            trp[:, b4 * P:(b4 + 1) * P],
                            xt[:, g * 512 + b4 * P:g * 512 + (b4 + 1) * P],
                            id32r)
                    copy_balanced(
                        g, xT[:, g * 4:(g + 1) * 4, tb * P:(tb + 1) * P],
                        trp.rearrange("p (b f) -> p b f", b=4))

        def kt_win(win):
            for p in range(DC):
                wh = [dma_wpan_half(wk, p, h) for h in range(2)]
                ps = ps512.tile([P, 512], F32, tag="ps512", name="psk")
                for dc in range(DC):
                    nc.tensor.matmul(
                        ps, wh[dc // 4][:, dc % 4, :],
                        xT[:, dc, win * 512:(win + 1) * 512],
                        start=(dc == 0), stop=(dc == DC - 1))
                gtok = c * 1024 + win * 512
                copy_balanced(p + win, kT[:, p, gtok:gtok + 512], ps)

        if c == 0:
            # x streams from DRAM at t=0: interleave kT windows with the
            # transposes so the PE has matmul work while x tiles land.
            # Window-1 x DMAs are issued right after window 0's so they land
            # before the window-0 kT matmuls finish.
            xts0 = [dma_x(0, tb, split=(4 if tb == 0 else 2))
                    for tb in range(4)]
            transp_win(0, xts0)
            kt_win(0)
            xts1 = [dma_x(0, tb) for tb in range(4, 8)]
            transp_win(1, xts1)
            kt_win(1)
        else:
            # x was prefetched during chunk-0 attention (8-deep ring covers
            # the whole chunk); transpose it all, then run kT panel-wise
            # with each Wk panel loaded only once.
            xts = [dma_x(1, tb) for tb in range(8)]
            transp_win(0, xts[0:4])
            transp_win(1, xts[4:8])
            for p in range(DC):
                wh = [dma_wpan_half(wk, p, h) for h in range(2)]
                for win in range(2):
                    ps = ps512.tile([P, 512], F32, tag="ps512", name="psk")
                    for dc in range(DC):
                        nc.tensor.matmul(
                            ps, wh[dc // 4][:, dc % 4, :],
                            xT[:, dc, win * 512:(win + 1) * 512],
                            start=(dc == 0), stop=(dc == DC - 1))
                    gtok = c * 1024 + win * 512
                    copy_balanced(p + win, kT[:, p, gtok:gtok + 512], ps)

        # -- qT pass (panel-wise, panel resident over both wins) --
        for p in range(DC):
            wh = [dma_wpan_half(wq, p, h) for h in range(2)]
            for win in range(2):
                ps = ps512.tile([P, 512], F32, tag="ps512", name="psq")
                for dc in range(DC):
                    nc.tensor.matmul(
                        ps, wh[dc // 4][:, dc % 4, :],
                        xT[:, dc, win * 512:(win + 1) * 512],
                        start=(dc == 0), stop=(dc == DC - 1))
                copy_balanced(p + win + 1,
                              qTc[:, p, win * 512:(win + 1) * 512], ps)

        # -- v pass: natural [tok, d] layout, xT stationary, Wv moving --
        for q4 in range(4):
            wvq = wvpool.tile([P, DC, 256], F32R, tag="wvq", name="wvq")
            nc.sync.dma_start(
                out=wvq,
                in_=wv[:, q4 * 256:(q4 + 1) * 256]
                .rearrange("(c p) f -> p c f", p=P).bitcast(F32R))
            for t in range(8):
                ps = ps512.tile([P, 512], F32, tag="ps512", name="psv")
                for dc in range(DC):
                    nc.tensor.matmul(
                        ps[:, 0:256], xT[:, dc, t * P:(t + 1) * P],
                        wvq[:, dc, :],
                        start=(dc == 0), stop=(dc == DC - 1))
                copy_balanced(q4 + t, vres[:, c * 8 + t, q4 * 256:(q4 + 1) * 256],
                              ps[:, 0:256])

        wvpool.release()
        wpool.release()
        xpool.release()

        # -- attention for the two 512-query chunks of this chunk --
        apool = tc.alloc_tile_pool(name="apool", bufs=1)
        expP = apool.tile([P, NBLK, 512], BF16, tag="expP")  # [m%128, mblk, n]

        for jj in range(2):
            j = 2 * c + jj
            # scores S^T[m, n] + exp, all key blocks mb for query chunk j
            for mb in range(4 * j + 4):
                il_p = mb - 4 * j  # >= 0 on the diagonal 512-chunk
                nlo = il_p * P if il_p >= 0 else 0
                w = 512 - nlo
                sS = ps512.tile([P, 512], F32, tag="ps512", name="sS")
                for dc in range(DC):
                    nc.tensor.matmul(
                        sS[:, 0:w], kT[:, dc, mb * P:(mb + 1) * P],
                        qTc[:, dc, jj * 512 + nlo:(jj + 1) * 512],
                        start=(dc == 0), stop=(dc == DC - 1))
                if il_p >= 0:
                    nc.vector.tensor_add(sS[:, 0:w], sS[:, 0:w],
                                         maskS[:, 0:w])
                nc.scalar.activation(expP[:, mb, nlo:512], sS[:, 0:w], EXP,
                                     scale=SCALE)
            # AV accumulation per 128-query block i (den rides in cols
            # 1024:1032 of the same PSUM tile via the ones8 matmul)
            for il in range(4):
                i = 4 * j + il
                Ops = psbig.tile([P, 1024], F32, tag="psbig", name="Ops")
                dreg = denb[:, i * 8:i * 8 + 8]
                for mb in range(i + 1):
                    st = (mb == 0)
                    sp = (mb == i)
                    lhsT = expP[:, mb, il * P:(il + 1) * P]
                    nc.tensor.matmul(Ops[:, 0:512], lhsT,
                                     vres[:, mb, 0:512], start=st, stop=sp)
                    nc.tensor.matmul(Ops[:, 512:1024], lhsT,
                                     vres[:, mb, 512:1024], start=st, stop=sp)
                    nc.tensor.matmul(dreg, lhsT, ones8, start=st, stop=sp)
                rcp = pden.tile([P, 1], F32, tag="rcp", name="rcp")
                nc.vector.reciprocal(rcp, dreg[:, 0:1])
                Osb = posb.tile([P, D], F32, tag="Osb", name="Osb")
                # finer pieces on the very last block shorten the kernel tail
                nsp = 4 if i == NBLK - 1 else 2
                for hf in range(nsp):
                    sl = slice(hf * (D // nsp), (hf + 1) * (D // nsp))
                    nc.vector.tensor_scalar_mul(Osb[:, sl], Ops[:, sl], rcp)
                    nc.sync.dma_start(out=out[i * P:(i + 1) * P, sl],
                                      in_=Osb[:, sl])

        apool.release()
        qpool.release()

    for pool in (posb, pden, xnat, pdenb, psbig, ps512, resid, const):
        pool.release()


_NC_CACHE = None


def _get_nc():
    global _NC_CACHE
    if _NC_CACHE is None:
        _NC_CACHE = build_program()
    return _NC_CACHE


def kernel(x, W_query, W_key, W_value):
    """Full causal attention: x [8, 2048, 1024] -> [8, 2048, 1024] (f32)."""
    nc = _get_nc()
    x = np.ascontiguousarray(np.asarray(x, dtype=np.float32))
    wq = np.ascontiguousarray(np.asarray(W_query, dtype=np.float32))
    wk = np.ascontiguousarray(np.asarray(W_key, dtype=np.float32))
    wv = np.ascontiguousarray(np.asarray(W_value, dtype=np.float32))
    n_cores = x.shape[0]
    in_maps = [
        {"x": x[b], "W_query": wq, "W_key": wk, "W_value": wv}
        for b in range(n_cores)
    ]
    res = bass_utils.run_bass_kernel_spmd(nc, in_maps, core_ids=list(range(n_cores)))
    return np.stack([res.results[b]["out"] for b in range(n_cores)])
